# revision 2
# baseline (speedup 1.0000x reference)
"""Blake2 soft-cipher Bass kernel for Trainium2 — fused custom-DVE version.

Design:
 - 8 cores data-parallel; per core CORE_ROWS=250880 rows in BLOCKS=2 blocks
   of [P=128, FDB=980] (each word plane), split into NH=2 half-streams of
   HF=490 elems for cross-stream engine pipelining.
 - All 16 working words live in one v region [P, 16*FDB]; word w half h at
   offset w*FDB + h*HF.  Message words interleaved in m tile (stride 16).
 - Round entries for v[8..15] (and round-0 v[0..7]) are IV constants folded
   into per-lane ops; no reset copies.
 - soft_xor combine, rot32 (2 ops), rot63 are single custom DVE ops.
 - Plain adds/subs are balanced between DVE and Pool (gpsimd) engines.
"""
import sys
sys.path.insert(0, "/opt/trn_rl_repo")
import math
import numpy as np
from concourse import bass, mybir
from concourse.tile import TileContext
from concourse.ap import AP
from concourse.bass_primitives_rust import SemaphoreHandle
from concourse.bass import _bass_rust
from concourse.library_overlay import lower_extended_insts

from concourse import dve_ops as _dve_ops
from concourse.dve_spec import (
    Spec, Src0, Src1, C0, C1, C2, Zero, One, Bin, AluOp,
    lower as _dve_lower, _has_src1, minn,
)
from concourse.dve_uop import DveOpSpec

A = mybir.AluOpType
F = mybir.ActivationFunctionType
DT = mybir.dt.float32

# ---------------------------------------------------------------- geometry
P = 128
FDB = 980          # elems per word-plane per block
NH = 5
_base, _rem = divmod(FDB, NH)
HFS = [_base + (1 if i < _rem else 0) for i in range(NH)]
HOFF = [sum(HFS[:i]) for i in range(NH)]
HF = HFS[0]        # max stream width (sizes scratch of widest stream)
BLOCKS = 2
BLOCK_ROWS = P * FDB          # 125440
CORE_ROWS = BLOCK_ROWS * BLOCKS
N_CORES = 8
PAD_ROWS = CORE_ROWS * N_CORES

ROUNDS = 10
_IV_INTS = [7640891576956012808, 13503953896175478587, 4354685564936845355,
            11912009170470909681, 5840696475078001361, 11170449401992604703,
            2270897969802886507, 6620516959819538809]
IV = (np.asarray(_IV_INTS, dtype=np.float32) / np.float32(2.0**64)).astype(np.float32)
STEEP = np.float32(10.0)
SC24 = float(2.0 ** -24)
SC16 = float(2.0 ** -16)


def f32(x):
    return np.float32(x)


def sig_const(z):
    return f32(1.0 / (1.0 + math.exp(-float(z))))


# ---------------------------------------------------------------- custom ops
def _register_dve_op(name, body):
    if name in _dve_ops._SUB_OPCODE_FOR_NAME:
        for o in _dve_ops.OPS:
            if o.name == name:
                return o
        raise RuntimeError(name)
    spec = Spec(body=body)
    row = max(_dve_ops._SUB_OPCODE_FOR_NAME.values(), default=0) + 1
    assert row < 0x20
    _dve_ops._SUB_OPCODE_FOR_NAME[name] = row
    uops = _dve_lower(spec, ver="v3")
    tmp = DveOpSpec(name=name, opcode=row, uops=uops, rd1_en=_has_src1(spec))
    op = _dve_ops.DveOp(name, spec, subdim=False, uops_sha={"v3": tmp.sha("v3")})
    _dve_ops.OPS.append(op)
    return op


def _build_ops():
    _A = One - Src1
    _T1 = Src0 * _A
    _B = One - Src0
    _T2 = Src1 * _B
    xorc = _register_dve_op("ANT_XORC",
                            One - ((One - _T1) * (One - _T2)))
    _cA = One - Src0
    _cT1 = C0 * _cA
    _cT2 = C1 * Src0
    xorc_c = _register_dve_op("ANT_XORC_C",
                              (_cT1 + _cT2) - (_cT1 * _cT2))
    _rU1 = Src0 * C0
    _rC = minn(_rU1, C1)
    _rR = _rC + C1
    _rI = C1 - _rR
    _rU4 = _rI + _rC
    _rT = Src0 * C2
    rot32a = _register_dve_op("ANT_ROT32A", _rT + _rU4)
    _wM = Bin(AluOp.IS_GE, Src0, Zero)
    rot32b = _register_dve_op("ANT_ROT32B", (Src0 + One) - _wM)
    _sM = Bin(AluOp.IS_GE, Src0, C0)
    _sF = (Src0 * C1) - _sM
    rot63 = _register_dve_op("ANT_ROT63", (Src0 * C2) + _sF)
    return xorc, xorc_c, rot32a, rot32b, rot63


XORC, XORC_C, ROT32A, ROT32B, ROT63 = _build_ops()

# ---------------------------------------------------------------- cost model
def ns_dve(E):
    return (E + 58) / 0.96


def ns_act(E):
    return (E + 222) / 1.2


def ns_pool_tt(E):
    return E / 0.42 / 1.2 + 95.0


def ns_pool_ts(E):
    return E / 0.60 / 1.2 + 95.0


# ---------------------------------------------------------------- values
class GV:
    """A 4-lane (or 8-lane) group value: per-lane element offsets into a
    tile ([P, *]) with an element stride, plus optional pending scale, or
    per-lane host constants."""

    __slots__ = ("get", "offs", "estride", "scale", "consts", "n")

    def __init__(self, get=None, offs=None, estride=1, scale=None,
                 consts=None, n=HF):
        self.get = get
        self.offs = offs
        self.estride = estride
        self.scale = scale
        self.consts = consts
        self.n = n

    @property
    def is_const(self):
        return self.consts is not None


def segs_of(offs):
    """Maximal constant-stride lane runs of one offset list."""
    runs = []
    i = 0
    L = len(offs)
    while i < L:
        j = i
        if i + 1 < L:
            d = offs[i + 1] - offs[i]
            if d > 0:
                j = i + 1
                while j + 1 < L and offs[j + 1] - offs[j] == d:
                    j += 1
        runs.append((i, j - i + 1))
        i = j + 1
    return runs


def joint_segs(gvs, per_lane=False):
    """Lane runs where every tensor operand has a constant lane stride."""
    L = None
    for g in gvs:
        if g is not None and not g.is_const:
            L = len(g.offs)
            break
    if L is None:
        L = 4
    if per_lane:
        return [(i, 1) for i in range(L)]
    cuts = {0, L}
    for g in gvs:
        if g is None or g.is_const:
            continue
        for (s, c) in segs_of(g.offs):
            cuts.add(s)
            cuts.add(s + c)
    cs = sorted(cuts)
    return [(cs[i], cs[i + 1] - cs[i]) for i in range(len(cs) - 1)]


# ---------------------------------------------------------------- program
class Prog:
    def __init__(self):
        self.nc = bass.Bass("TRN2")
        self.est = {"dve": 0.0, "act": 0.0, "pool": 0.0}
        self.cur = None  # current stream thunk list
        self.pre = []    # prologue thunks (bias memsets)
        self.bias_pool = None
        self._bias = {}
        self._cgv = {}

    def const_gv(self, tag, vals, hf):
        """[P, 4*hf] tile with vals[l] broadcast in lane l (memset x4 in
        prologue). Returns GV."""
        key = (tag, hf, tuple(float(v) for v in vals))
        if key not in self._cgv:
            tg = f"{tag}_{hf}"
            t = self.bias_pool.tile([P, 4 * hf], DT, tag=tg, name=tg)
            for l, v in enumerate(vals):
                self.pre.append(
                    lambda t=t, v=float(v), l=l, hf=hf:
                    self.nc.vector.memset(t[:][:, l * hf:(l + 1) * hf], v))
            self._cgv[key] = GV(get=lambda t=t: t[:],
                                offs=[0, hf, 2 * hf, 3 * hf], n=hf)
        return self._cgv[key]

    def bias_ap(self, val):
        """Column of the shared bias tile for a bias value (cached)."""
        v = float(f32(val))
        if v not in self._bias:
            i = len(self._bias)
            assert i < 16
            t = self.bias_tile
            self.pre.append(lambda t=t, v=v, i=i:
                            self.nc.vector.memset(t[:][:, i:i + 1], v))
            self._bias[v] = i
        i = self._bias[v]
        t = self.bias_tile
        return lambda t=t, i=i: t[:][:, i:i + 1]

    def emit(self, fn, eng="sp", cost=100.0):
        self.cur.append((fn, eng, cost))

    # ---- AP construction
    def ap_of(self, gv, s, c):
        """AP for lanes [s, s+c) of tensor gv."""
        base = gv.get()
        off = gv.offs[s]
        pd = list(base.ap[0])
        if c == 1:
            dims = [pd, [gv.estride, gv.n]]
        else:
            d = gv.offs[s + 1] - gv.offs[s]
            if gv.estride == 1 and d == gv.n:
                dims = [pd, [1, c * gv.n]]
            else:
                dims = [pd, [d, c], [gv.estride, gv.n]]
        return AP(base.tensor, base.offset + off, dims)

    # ---- engine emitters (thunked)
    def tt(self, out, a, b, op, pool_ok=True):
        for (s, c) in joint_segs([out, a, b]):
            E = c * out.n
            use_pool = False
            if pool_ok and op in (A.add, A.subtract, A.mult):
                if (self.est["pool"] + ns_pool_tt(E)
                        < self.est["dve"] + ns_dve(E)):
                    use_pool = True
            if use_pool:
                self.est["pool"] += ns_pool_tt(E)
                eng, en, cn = self.nc.gpsimd, "pool", ns_pool_tt(E)
            else:
                self.est["dve"] += ns_dve(E)
                eng, en, cn = self.nc.vector, "dve", ns_dve(E)
            oap, aap, bap = (self.ap_of(out, s, c), self.ap_of(a, s, c),
                             self.ap_of(b, s, c))
            self.emit(lambda e=eng, o=oap, x=aap, y=bap, op_=op:
                      e.tensor_tensor(o, x, y, op=op_), en, cn)

    def stt(self, out, in0, scalars, in1, op0, op1):
        """out = op1(op0(in0, scalar), in1); scalars: float or per-lane list."""
        per_lane = isinstance(scalars, (list, np.ndarray))
        for (s, c) in joint_segs([out, in0, in1], per_lane=per_lane):
            E = c * out.n
            self.est["dve"] += ns_dve(E)
            sc = float(scalars[s]) if per_lane else float(scalars)
            oap, aap, bap = (self.ap_of(out, s, c), self.ap_of(in0, s, c),
                             self.ap_of(in1, s, c))
            self.emit(lambda o=oap, x=aap, y=bap, sc_=sc, a0=op0, a1=op1:
                      self.nc.vector.scalar_tensor_tensor(o, x, sc_, y,
                                                          op0=a0, op1=a1),
                      "dve", ns_dve(E))

    def sig(self, out, in_gv, bias, scale):
        """out = sigmoid(in*scale + bias); bias: float or per-lane list."""
        per_lane = isinstance(bias, (list, np.ndarray))
        for (s, c) in joint_segs([out, in_gv], per_lane=per_lane):
            E = c * out.n
            self.est["act"] += ns_act(E)
            b = float(bias[s]) if per_lane else float(bias)
            bget = self.bias_ap(b)
            oap, iap = self.ap_of(out, s, c), self.ap_of(in_gv, s, c)
            self.emit(lambda o=oap, i=iap, bg=bget, sc=float(scale):
                      self.nc.scalar.activation(o, i, F.Sigmoid,
                                                bias=bg(), scale=sc),
                      "act", ns_act(E))

    def custom(self, op, out, in0, in1=None, s0=0.0, s1=0.0, imm2=0.0,
               consts_per_lane=None):
        """consts_per_lane: list of (s0, s1) per lane forces per-lane split."""
        per_lane = consts_per_lane is not None
        for (s, c) in joint_segs([out, in0, in1], per_lane=per_lane):
            E = c * out.n
            self.est["dve"] += ns_dve(E)
            if per_lane:
                s0v, s1v = consts_per_lane[s]
            else:
                s0v, s1v = s0, s1
            oap = self.ap_of(out, s, c)
            iap = self.ap_of(in0, s, c)
            jap = self.ap_of(in1, s, c) if in1 is not None else None
            def th(o=oap, i=iap, j=jap, op_=op, a=float(s0v), b=float(s1v),
                   c_=float(imm2)):
                if j is None:
                    self.nc.vector._custom_dve(op_, out=o, in0=i,
                                               s0=a, s1=b, imm2=c_)
                else:
                    self.nc.vector._custom_dve(op_, out=o, in0=i, in1=j,
                                               s0=a, s1=b, imm2=c_)
            self.emit(th, "dve", ns_dve(E))


# ---------------------------------------------------------------- stream ctx
class Stream:
    """Per (block, half) emission context."""

    def __init__(self, pg, blk, h, v_get, m_get, scr_alloc):
        self.pg = pg
        self.blk = blk
        self.h = h
        self.hf = HFS[h]
        self.v_get = v_get
        self.m_get = m_get
        self.scr = scr_alloc   # () -> GV (fresh [P, 4*hf] scratch, 4 lanes)

    # v-backed group value for given word list
    def vg(self, words, scale=None):
        offs = [w * self.hf for w in words]
        return GV(get=self.v_get, offs=offs, scale=scale, n=self.hf)

    def mg(self, words):
        offs = [w for w in words]
        return GV(get=self.m_get, offs=offs, estride=16, n=self.hf)

    # ------------ math ops
    def soft_add(self, x, y, out=None, ctile=None):
        pg = self.pg
        if x.is_const and y.is_const:
            r = []
            for cx, cy in zip(x.consts, y.consts):
                s = f32(cx + cy)
                r.append(f32(s - sig_const(STEEP * (s - f32(1.0)))))
            return GV(consts=r)
        if x.is_const or y.is_const:
            cv = x.consts if x.is_const else y.consts
            t = y if x.is_const else x
            assert t.scale is None
            if ctile is not None:
                cgv = pg.const_gv(ctile, cv, t.n)
                s_ = self.scr()
                pg.tt(s_, t, cgv, A.add)
                w = self.scr_w()
                pg.sig(w, s_, -10.0, 10.0)
                dst = out if out is not None else self.scr()
                pg.tt(dst, s_, w, A.subtract,
                      pool_ok=self.scr_w is self.scr)
                return dst
            w = self.scr()
            pg.sig(w, t, [f32(10.0 * c - 10.0) for c in cv], 10.0)
            dst = out if out is not None else self.scr()
            pg.stt(dst, t, [float(c) for c in cv], w, A.add, A.subtract)
            return dst
        s = self.scr()
        if x.scale is not None or y.scale is not None:
            sv, ov = (x, y) if x.scale is not None else (y, x)
            assert ov.scale is None
            pg.stt(s, sv, float(sv.scale), ov, A.mult, A.add)
        else:
            pg.tt(s, x, y, A.add)
        w = self.scr_w()
        pg.sig(w, s, -10.0, 10.0)
        dst = out if out is not None else self.scr()
        pg.tt(dst, s, w, A.subtract)
        return dst

    def sig_of(self, v):
        """Precompute sigmoid(10*(v-0.5)) into scratch (side input of xor)."""
        ts = self.scr_pre()
        sc = 10.0 * (v.scale if v.scale is not None else 1.0)
        self.pg.sig(ts, v, -5.0, sc)
        return ts

    def soft_xor(self, x, y, out=None, pre_x=None, xtile=None):
        pg = self.pg
        dst = out if out is not None else self.scr()
        if x.is_const and y.is_const:
            r = []
            for cx, cy in zip(x.consts, y.consts):
                xs = sig_const(STEEP * (cx - f32(0.5)))
                ys = sig_const(STEEP * (cy - f32(0.5)))
                t1 = f32(xs * f32(1.0 - ys))
                t2 = f32(f32(1.0 - xs) * ys)
                v = f32(f32(t1 + t2) - f32(t1 * t2))
                r.append(f32(min(max(v, 0.0), 1.0)))
            return GV(consts=r)
        if x.is_const or y.is_const:
            cv = x.consts if x.is_const else y.consts
            t = y if x.is_const else x
            ts = self.scr()
            sc = 10.0 * (t.scale if t.scale is not None else 1.0)
            pg.sig(ts, t, -5.0, sc)
            sig_c = [sig_const(STEEP * (f32(c) - f32(0.5))) for c in cv]
            if xtile is not None:
                xs_c = pg.const_gv(xtile, sig_c, t.n)
                pg.custom(XORC, dst, xs_c, ts)
            else:
                cl = [(float(c_), float(f32(1.0 - c_))) for c_ in sig_c]
                pg.custom(XORC_C, dst, ts, consts_per_lane=cl)
            return dst
        if pre_x is not None:
            xs = pre_x
        else:
            xs = self.scr()
            pg.sig(xs, x, -5.0,
                   10.0 * (x.scale if x.scale is not None else 1.0))
        ys = self.scr()
        pg.sig(ys, y, -5.0, 10.0 * (y.scale if y.scale is not None else 1.0))
        pg.custom(XORC, dst, xs, ys)
        return dst

    def rot32(self, x, out=None):
        pg = self.pg
        c2 = self.scr()
        pg.custom(ROT32A, c2, x, s0=float(2.0 ** 32), s1=float(2.0 ** 23),
                  imm2=float(2.0 ** -32))
        dst = out if out is not None else self.scr()
        pg.custom(ROT32B, dst, c2)
        return dst

    def rot63(self, x, out=None):
        dst = out if out is not None else self.scr()
        self.pg.custom(ROT63, dst, x, s0=0.5, s1=2.0, imm2=float(2.0 ** -63))
        return dst

    # ------------ one G bundle over 4 lanes
    def G(self, av, bv, cv, dv, mx, my, wa, wb, wc, wd):
        """av..dv: entry GVals; wa..wd: per-lane v word lists for outputs.
        Returns (a4, b4, c2, d3) GVals (v-backed)."""
        va = lambda: self.vg(wa)
        vb = lambda: self.vg(wb)
        vc = lambda: self.vg(wc)
        vd = lambda: self.vg(wd)
        # side sigmoids of entry values up front: ACT work independent of
        # the serial add-chain
        pre_xd = pre_xb = None
        a1 = self.soft_add(av, bv)
        a2 = self.soft_add(a1, mx, out=va())
        d1 = self.soft_xor(dv, a2, out=vd(), pre_x=pre_xd,
                           xtile="xsdiv" if dv.is_const else None)
        d2 = self.rot32(d1, out=vd())
        c1 = self.soft_add(cv, d2, out=vc(),
                           ctile="civ" if cv.is_const else None)
        b1 = self.soft_xor(bv, c1, out=vb(), pre_x=pre_xb)
        b1s = GV(get=b1.get, offs=b1.offs, scale=f32(SC24), n=b1.n)
        a3 = self.soft_add(a2, b1s, out=va())
        a4 = self.soft_add(a3, my, out=va())
        d3 = self.soft_xor(d2, a4, out=vd())
        d3s = GV(get=d3.get, offs=d3.offs, scale=f32(SC16), n=d3.n)
        c2 = self.soft_add(c1, d3s, out=vc())
        b3 = self.soft_xor(b1s, c2)
        b4 = self.rot63(b3, out=vb())
        return a4, b4, c2, GV(get=d3.get, offs=d3.offs, scale=f32(SC16), n=d3.n)


G_SCHEDULE = [
    (0, 4, 8, 12, 0, 1), (1, 5, 9, 13, 2, 3), (2, 6, 10, 14, 4, 5),
    (3, 7, 11, 15, 6, 7),
    (0, 5, 10, 15, 8, 9), (1, 6, 11, 12, 10, 11), (2, 7, 8, 13, 12, 13),
    (3, 4, 9, 14, 14, 15),
]


def build_program():
    pg = Prog()
    nc = pg.nc
    msg = nc.declare_dram_parameter("message", [CORE_ROWS, 16], DT,
                                    isOutput=False)
    outp = nc.declare_dram_parameter("out", [CORE_ROWS, 8], DT, isOutput=True)
    with TileContext(nc) as tc:
        with (
            tc.tile_pool(name="persist", bufs=1) as pp,
            tc.tile_pool(name="scrp", bufs=1) as sp,
            tc.tile_pool(name="psump", bufs=1, space="PSUM") as qp,
        ):
            m_tiles = [pp.tile([P, 16 * HFS[h]], DT, tag=f"m{h}",
                                name=f"m{h}") for h in range(NH)]
            v_tiles = [pp.tile([P, 16 * HFS[h]], DT, tag=f"v{h}",
                               name=f"v{h}") for h in range(NH)]
            pg.bias_pool = pp
            pg.bias_tile = pp.tile([P, 16], DT, tag="biases", name="biases")

            scr_cells = {}

            import os as _os
            _RING = 5
            def make_scr(h, tag=None, bufs=None, pool=None):
                bufs = bufs if bufs is not None else _RING
                tag = tag or f"g{h}"
                pool = pool or sp
                hf = HFS[h]
                def alloc():
                    t = pool.tile([P, 4 * hf], DT, tag=tag, name=tag,
                                  bufs=bufs)
                    return GV(get=lambda t=t: t[:],
                              offs=[0, hf, 2 * hf, 3 * hf], n=hf)
                return alloc

            streams_done = []
            # build one continuous list per half-stream spanning both blocks
            # (m DMA at each block head, out DMA after each block tail), then
            # merge once — no cross-stream sync at the block seam
            lists = []
            for h in range(NH):
                pg.cur = []
                st = Stream(pg, 0, h,
                            (lambda t=v_tiles[h]: t[:]),
                            (lambda t=m_tiles[h]: t[:]),
                            make_scr(h))
                st.scr_pre = make_scr(h, tag=f"p{h}", bufs=2)
                st.scr_w = st.scr
                for blk in range(BLOCKS):
                    r0 = blk * BLOCK_ROWS
                    mb = msg[:, :]
                    in_ap = AP(mb.tensor, (r0 + HOFF[h]) * 16,
                               [[FDB * 16, P], [1, 16 * HFS[h]]])
                    pg.emit(lambda o=m_tiles[h], i=in_ap:
                            nc.sync.dma_start(out=o[:], in_=i),
                            "sp", 700.0)
                    st.blk = blk
                    emit_stream(pg, st)
                    ob = outp[:, :]
                    out_ap = AP(ob.tensor, (r0 + HOFF[h]) * 8,
                                [[FDB * 8, P], [1, 8 * HFS[h]]])
                    pg.emit(lambda o=out_ap, i=v_tiles[h], n8=8 * HFS[h]:
                            nc.sync.dma_start(out=o, in_=i[:][:, 0:n8]),
                            "sp", 700.0)
                lists.append(pg.cur)
            for fn in pg.pre:
                fn()
            pg.pre = []
            sched = "rr"
            if sched == "lst":
                pos = [0] * NH
                ready = [0.0] * NH
                avail = {"dve": 0.0, "act": 0.0, "pool": 0.0, "sp": 0.0}
                while True:
                    best, bt = None, None
                    for h in range(NH):
                        if pos[h] >= len(lists[h]):
                            continue
                        _, en, cn = lists[h][pos[h]]
                        t = max(ready[h], avail[en])
                        if bt is None or t < bt:
                            best, bt = h, t
                    if best is None:
                        break
                    fn, en, cn = lists[best][pos[best]]
                    fn()
                    pos[best] += 1
                    avail[en] = bt + cn
                    ready[best] = bt + cn
            else:
                LAG = 21
                BURST = 1
                pos = [0] * NH
                started = [h * LAG for h in range(NH)]
                total = 0
                while any(pos[h] < len(lists[h]) for h in range(NH)):
                    moved = False
                    for h in range(NH):
                        for _ in range(BURST):
                            if total >= started[h] and pos[h] < len(lists[h]):
                                lists[h][pos[h]][0]()
                                pos[h] += 1
                                total += 1
                                moved = True
                    if not moved:
                        total += 1
    hoist_excess_waits(nc)
    lower_extended_insts(nc)
    return nc, pg


def emit_stream(pg, st):
    """Emit all rounds for one (block, half) stream."""
    h = st.h
    for rnd in range(ROUNDS):
        # ---- group 1: words (l, 4+l, 8+l, 12+l); c,d entries = IV consts
        wa = [0, 1, 2, 3]
        wb = [4, 5, 6, 7]
        wc = [8, 9, 10, 11]
        wd = [12, 13, 14, 15]
        if rnd == 0:
            av = GV(consts=[IV[0], IV[1], IV[2], IV[3]])
            bv = GV(consts=[IV[4], IV[5], IV[6], IV[7]])
        else:
            av = st.vg(wa)
            bv = st.vg(wb)
        cv = GV(consts=[IV[0], IV[1], IV[2], IV[3]])
        dv = GV(consts=[IV[4], IV[5], IV[6], IV[7]])
        mx = st.mg([0, 2, 4, 6])
        my = st.mg([1, 3, 5, 7])
        a4, b4, c2, d3 = st.G(av, bv, cv, dv, mx, my, wa, wb, wc, wd)
        # ---- group 2: lane l -> G(l, 4+(l+1)%4, 8+(l+2)%4, 12+(l+3)%4)
        wa2 = [0, 1, 2, 3]
        wb2 = [5, 6, 7, 4]
        wc2 = [10, 11, 8, 9]
        wd2 = [15, 12, 13, 14]
        av2 = st.vg(wa2)
        bv2 = st.vg(wb2)
        cv2 = st.vg(wc2)
        dv2 = st.vg(wd2, scale=f32(SC16))
        mx2 = st.mg([8, 10, 12, 14])
        my2 = st.mg([9, 11, 13, 15])
        st.G(av2, bv2, cv2, dv2, mx2, my2, wa2, wb2, wc2, wd2)
        # ---- final xor: state[j] = xor(v[j], v[8+j])
        last = rnd == ROUNDS - 1

        def out_gv(j0):
            offs = [j for j in range(j0, j0 + 4)]
            return GV(get=st.v_get, offs=offs, estride=8, n=st.hf)
        xs_a = st.scr()
        xs_b = st.scr()
        ys_c = st.scr()
        ys_d = st.scr()
        pg.sig(xs_a, st.vg([0, 1, 2, 3]), -5.0, 10.0)
        pg.sig(xs_b, st.vg([4, 5, 6, 7]), -5.0, 10.0)
        pg.sig(ys_c, st.vg([8, 9, 10, 11]), -5.0, 10.0)
        pg.sig(ys_d, st.vg([12, 13, 14, 15]), -5.0, 10.0 * SC16)
        pg.custom(XORC, out_gv(0) if last else st.vg([0, 1, 2, 3]),
                  xs_a, ys_c)
        pg.custom(XORC, out_gv(4) if last else st.vg([4, 5, 6, 7]),
                  xs_b, ys_d)


def hoist_excess_waits(nc, max_waits=1):
    n_hoisted = 0
    for f in nc.m.functions:
        for blk in f.blocks:
            need = False
            for inst in blk.instructions:
                si = inst.sync_info
                if si is not None and len(si.on_wait) > max_waits:
                    need = True
                    break
            if not need:
                continue
            newl = []
            for inst in blk.instructions:
                si = inst.sync_info
                if si is not None and len(si.on_wait) > max_waits:
                    conds = list(si.on_wait)
                    keep = conds[-max_waits:]
                    for c in conds[:-max_waits]:
                        nop = mybir.InstNoOp(
                            name=nc.get_next_instruction_name(), ins=[], outs=[])
                        nop.engine = inst.engine
                        _bass_rust.wait_op(
                            nop, SemaphoreHandle(c.ant_name, c.id),
                            c.wait_value, "sem-ge", False)
                        newl.append(nop)
                        n_hoisted += 1
                    inst.sync_info = mybir.SyncInfo(
                        on_wait=keep, on_update=list(si.on_update))
                newl.append(inst)
            blk.instructions = newl
    return n_hoisted


# ----------------------------------------------------------------- entry
_cache = {}


def _get_nc():
    if "nc" not in _cache:
        nc, pg = build_program()
        _cache["nc"] = nc
        _cache["pg"] = pg
    return _cache["nc"]


def kernel(message, _trace=False):
    """Full (2000000, 16) f32 in -> (2000000, 8) f32 out, 8-core DP."""
    from concourse.bass_utils import run_bass_kernel_spmd
    msg = np.ascontiguousarray(np.asarray(message, dtype=np.float32))
    nc = _get_nc()
    pad = PAD_ROWS - msg.shape[0]
    msgp = (np.concatenate([msg, np.zeros((pad, 16), np.float32)])
            if pad > 0 else msg)
    shards = msgp.reshape(N_CORES, CORE_ROWS, 16)
    in_maps = [{"message": shards[i]} for i in range(N_CORES)]
    kw = dict(trace=True) if _trace else {}
    res = run_bass_kernel_spmd(nc, in_maps, core_ids=list(range(N_CORES)), **kw)
    out = np.concatenate([res.results[i]["out"] for i in range(N_CORES)],
                         axis=0)
    if _trace:
        _cache["last_result"] = res
    return out[: msg.shape[0]]
